# revision 37
# baseline (speedup 1.0000x reference)
"""Multi-head attention (B=4, S=2048, D=1024, H=16, hd=64) with RoPE on 8 trn2 cores.

Sharding: core c handles batch b=c//2, head-group hg=c%2 (8 heads, 512 features).
Each core computes y_partial.T = Wo[:, fslice] @ ctx.T for its heads; the host
sums the two partials per batch and adds bo.

v4: paired score matmuls + barrier-free repeats. The two heads of a feature
pair contract only 64 partitions each, so their score matmuls are emitted
adjacently at PE tile_position (0,0) / (64,0): on hardware the two 64-row
streams run concurrently in the array halves, roughly halving score time.
Per (pair, q-block) slot both heads' scores for one k-chunk land in one
[128, 2, 512] psum tile and one ACT exp writes es[:, kc, 0:2, :]. PV of the
previous slot plus V/QK-projection/out-proj fillers keep the PE busy between
score chunks. The RoPE partition swap runs on the DVE (4 shifted tensor_
copies), not the PE. All pools and fixed tiles live outside the repeat loop
(out-proj psum shares the projection psum tag), so repeats pipeline without
pool-close barriers; only the DMAs and the instruction stream are per-rep.

Device layout:
  x_sb  [128, 8, 2048]  x.T by d-chunk (p=partition within d-chunk)
  qT/kT [128, 2048] bf16 per pair (rows 0:64 head0, 64:128 head1), RoPE applied
  vt    [128, 8, 65] bf16 per k-chunk: V rows + ones column (softmax denom)
  scores.T psum [k, 2, q] -> exp (ACT, bias -8, scale 1/8) -> es bf16
  es    [128, 16, 2, 512] per (pair, qb) slot (head0/head1 interleaved)
  PV: ctx.T[65, q] = vt^T @ es[:, kc, h, :] (row 64 = denominator)
  out:  y.T[e, q] = wo^T @ (ctxU * rden)
"""

import numpy as np

import concourse.mybir as mybir
import concourse.tile as tile
from concourse import bacc
from concourse.bass_utils import run_bass_kernel_spmd

F32 = mybir.dt.float32
BF16 = mybir.dt.bfloat16
AF = mybir.ActivationFunctionType
ADD = mybir.AluOpType.add
MULT = mybir.AluOpType.mult

B, S, D, H = 4, 2048, 1024, 16
HD = D // H            # 64
NCORES = 8
FC = D // 2            # 512 features (8 heads) per core
NH = FC // HD          # 8 heads per core (4 pairs)
NDC = D // 128         # 8 d-chunks
NFC = FC // 128        # 4 f-chunks (pairs)
NKC = S // 128         # 16 k-chunks
QB = 512               # q-block width
NQB = S // QB          # 4
EXP_BIAS = -8.0
SCALE = 1.0 / np.sqrt(HD)


def build_kernel(dump=False, repeat=1):
    nc = bacc.Bacc("TRN2", debug=False)

    xp = nc.dram_tensor("xp", [128, NDC, S], BF16, kind="ExternalInput")
    wq = nc.dram_tensor("wq", [128, NDC, FC], BF16, kind="ExternalInput")
    wk = nc.dram_tensor("wk", [128, NDC, FC], BF16, kind="ExternalInput")
    wv = nc.dram_tensor("wv", [128, NDC, FC], BF16, kind="ExternalInput")
    wo = nc.dram_tensor("wo", [128, NFC, D], BF16, kind="ExternalInput")
    bq = nc.dram_tensor("bq", [128, NFC], F32, kind="ExternalInput")
    bk = nc.dram_tensor("bk", [128, NFC], F32, kind="ExternalInput")
    bvf = nc.dram_tensor("bvf", [128, FC], BF16, kind="ExternalInput")
    c2 = nc.dram_tensor("c2", [128, S], BF16, kind="ExternalInput")
    s2 = nc.dram_tensor("s2", [128, S], BF16, kind="ExternalInput")
    yT = nc.dram_tensor("yT", [D, S], BF16, kind="ExternalOutput")

    with tile.TileContext(nc) as tc, \
         tc.tile_pool(name="const", bufs=1) as const, \
         tc.tile_pool(name="big", bufs=1) as big, \
         tc.tile_pool(name="esp", bufs=1) as esp, \
         tc.tile_pool(name="sbA", bufs=1) as sbA, \
         tc.tile_pool(name="bps", bufs=1, space="PSUM") as bps, \
         tc.tile_pool(name="qkps", bufs=1, space="PSUM") as qkps:
        c2_sb = const.tile([128, S], BF16, name="c2_sb")
        s2_sb = const.tile([128, S], BF16, name="s2_sb")
        bqs = const.tile([128, NFC], F32, name="bqs")
        bks = const.tile([128, NFC], F32, name="bks")
        bvs = const.tile([128, FC], BF16, name="bvs")
        wo_sb = const.tile([128, NFC, D], BF16, name="wo_sb")
        ebias = const.tile([128, 1], F32, name="ebias")
        nc.vector.memset(ebias, EXP_BIAS)

        vt = [big.tile([128, NH, HD + 1], BF16, name=f"vt{k}")
              for k in range(NKC)]
        # ctxU/den are split per q-block: the tile framework's dependency
        # tracking is per-tile, so a single [128, S] tile would serialize
        # this q-block's normalize against later drains of other q-blocks.
        ctxU = [[big.tile([128, QB], BF16, name=f"ctxU{i}_{j}")
                 for j in range(NQB)] for i in range(NFC)]
        # engine APs may only start at partitions {0,32,64,96}: spread the 8
        # denominator rows over two tiles at those bases; unused rows stay 1.0
        # so the full-tile reciprocal remains finite (drains rewrite only the
        # 4 live rows, so the memset is needed once, not per rep).
        den = [[big.tile([128, QB], BF16, name=f"den{i}_{j}")
                for j in range(NQB)] for i in range(2)]
        for i in range(2):
            for j in range(NQB):
                nc.vector.memset(den[i][j], 1.0)
        for kc in range(NKC):
            nc.vector.memset(vt[kc][:, :, HD:HD + 1], 1.0)

        x_sb = sbA.tile([128, NDC, S], BF16, name="x_sb")
        wq_sb = sbA.tile([128, NDC, FC], BF16, name="wq_sb")
        wk_sb = sbA.tile([128, NDC, FC], BF16, name="wk_sb")
        wv_sb = sbA.tile([128, NDC, FC], BF16, name="wv_sb")

        yT_r = yT[:].rearrange("(c p) s -> c p s", p=128)

        nc.gpsimd.dma_start(out=bqs, in_=bq[:])
        nc.gpsimd.dma_start(out=bks, in_=bk[:])
        nc.gpsimd.dma_start(out=bvs, in_=bvf[:])
        nc.gpsimd.dma_start(out=c2_sb, in_=c2[:])
        nc.gpsimd.dma_start(out=s2_sb, in_=s2[:])

        for _rep in range(repeat):
            _build_rep(nc, locals())

    nc.finalize()
    return nc


def _build_rep(nc, env):
    """Emit one repetition's instruction stream (DMAs + full pipeline)."""
    g = env
    const, big, esp, sbA, bps, qkps = (g[k] for k in
                                       ("const", "big", "esp", "sbA", "bps",
                                        "qkps"))
    c2_sb, s2_sb, bqs, bks, bvs, wo_sb, ebias = (g[k] for k in
        ("c2_sb", "s2_sb", "bqs", "bks", "bvs", "wo_sb", "ebias"))
    vt, ctxU, den = g["vt"], g["ctxU"], g["den"]
    x_sb, wq_sb, wk_sb, wv_sb = (g[k] for k in
                                 ("x_sb", "wq_sb", "wk_sb", "wv_sb"))
    xp, wq, wk, wv, wo, bq, bk, bvf, c2, s2 = (g[k] for k in
        ("xp", "wq", "wk", "wv", "wo", "bq", "bk", "bvf", "c2", "s2"))
    yT_r = g["yT_r"]

    # Startup DMA is latency-critical: interleave the sync and scalar
    # queues so wq/wk and the early x slabs land in parallel (the scalar
    # engine has no work until the first exp ~25us in). wv rides the
    # otherwise-idle gpsimd queue for the V fillers in the first slot.
    XSLAB = S // NDC
    nc.sync.dma_start(out=wq_sb, in_=wq[:])
    nc.scalar.dma_start(out=wk_sb, in_=wk[:])
    for i in range(NDC):
        ssl = slice(i * XSLAB, (i + 1) * XSLAB)
        eng = nc.sync if i % 2 == 0 else nc.scalar
        eng.dma_start(out=x_sb[:, :, ssl], in_=xp[:][:, :, ssl])
    nc.gpsimd.dma_start(out=wv_sb[:, 0:NDC // 2, :],
                        in_=wv[:][:, 0:NDC // 2, :])
    nc.gpsimd.dma_start(out=wv_sb[:, NDC // 2:NDC, :],
                        in_=wv[:][:, NDC // 2:NDC, :])
    nc.gpsimd.dma_start(out=wo_sb, in_=wo[:])

    # kT rotates with depth 2: pair p+1 is produced by fillers while pair p
    # is being consumed. qT is split per q-block (slot (p, qb) reads only
    # its own slice, and the per-tile dependency tracking would otherwise
    # force every q-subunit of pair p to finish before slot (p, 0)), so the
    # late q-subunits of the last pair can fill the otherwise-starved
    # (NFC-1, qb) slots.
    qkT = {}

    def get_qkT(t_idx, fc, sb=None):
        if t_idx == 1:
            key = (1, fc)
            if key not in qkT:
                qkT[key] = big.tile([128, S], BF16, name=f"kTr{fc}",
                                    tag="kTr", bufs=2)
            return qkT[key]
        key = (0, fc, sb)
        if key not in qkT:
            qkT[key] = big.tile([128, QB], BF16, name=f"qTr{fc}_{sb}",
                                tag="qTr", bufs=8)
        return qkT[key]

    def qk_subunit(t_idx, fc, sb):
        """One (q|k, fc, sb) projection+RoPE piece: 8 matmuls + 7 DVE."""
        w_t = wq_sb if t_idx == 0 else wk_sb
        bias_t = bqs if t_idx == 0 else bks
        out_t = get_qkT(t_idx, fc, sb)
        ssl = slice(sb * QB, (sb + 1) * QB)
        osl = ssl if t_idx == 1 else slice(0, QB)
        pp = qkps.tile([128, QB], F32, name="pp", tag="pp", bufs=2)
        for d in range(NDC):
            nc.tensor.matmul(
                pp, w_t[:, d, fc * 128:(fc + 1) * 128],
                x_sb[:, d, ssl], start=(d == 0), stop=(d == NDC - 1))
        praw = sbA.tile([128, QB], BF16, name="praw", tag="praw", bufs=1)
        nc.vector.tensor_scalar(
            praw, pp, bias_t[:, fc:fc + 1], None, op0=ADD)
        # RoPE half-rotation: the partition swap (32-blocks within each
        # 64-row head) is four DVE copies with shifted bases (tensor_tensor
        # requires equal SBUF base partitions, tensor_copy does not), then
        # in-place same-base multiplies — instead of a PE permute matmul:
        # the DVE has slack, the PE is the critical engine.
        prsw = sbA.tile([128, QB], BF16, name="prsw", tag="prsw", bufs=1)
        for a, b in ((0, 32), (32, 0), (64, 96), (96, 64)):
            nc.vector.tensor_copy(prsw[a:a + 32, :], praw[b:b + 32, :])
        nc.vector.tensor_tensor(prsw, prsw, s2_sb[:, ssl], op=MULT)
        nc.vector.tensor_tensor(out_t[:, osl], praw, c2_sb[:, ssl], op=MULT)
        nc.vector.tensor_tensor(out_t[:, osl], out_t[:, osl], prsw, op=ADD)

    def qk_pair(p):
        # k-subunits first: all of kT is read from slot (p, 0), while the
        # qT slice for q-block sb isn't needed until slot (p, sb).
        return ([(1, p, sb) for sb in range(NQB)]
                + [(0, p, sb) for sb in range(NQB)])

    def v_subunit(sc):
        psv = qkps.tile([128, FC], F32, name="psv", tag="pp", bufs=2)
        for d in range(NDC):
            nc.tensor.matmul(
                psv, x_sb[:, d, sc * 128:(sc + 1) * 128],
                wv_sb[:, d, :], start=(d == 0), stop=(d == NDC - 1))
        nc.vector.tensor_tensor(
            vt[sc][:, :, 0:HD],
            psv.rearrange("p (h e) -> p h e", e=HD),
            bvs.rearrange("p (h e) -> p h e", e=HD), op=ADD)

    # ---------------- attention ----------------
    out_emitted = [False] * NQB

    def out_proj_chunks(qb, tail=False):
        """Out-projection for one q-block as 8 filler pieces (cost ~1us).

        In the tail (no other PE work) the 2-buffer op rotation exposes
        the matmul->CAST->psum-free latency; alternating with the idle sc
        tag doubles the buffers in rotation.
        """
        qsl = slice(qb * QB, (qb + 1) * QB)

        def mk(ec):
            def f():
                # tail chunks rotate over the idle sc/ctx psum tags, keeping
                # the pp tag free so the next rep's projections start clean
                if tail and ec % 2 == 1:
                    sc_t = bps.tile([128, 2, QB], F32, name="sc",
                                    tag="sc", bufs=2)
                    op = sc_t[:, 0, :]
                elif tail:
                    op = bps.tile([128, QB], F32, name="ctx", tag="ctx",
                                  bufs=2)
                else:
                    op = qkps.tile([128, QB], F32, name="op", tag="pp",
                                   bufs=2)
                for fc in range(NFC):
                    nc.tensor.matmul(
                        op, wo_sb[:, fc, ec * 128:(ec + 1) * 128],
                        ctxU[fc][qb],
                        start=(fc == 0), stop=(fc == NFC - 1))
                ysb = big.tile([128, QB], BF16, name="ysb", tag="ysb",
                               bufs=2)
                if tail and ec % 2 == 1:
                    # ACT is idle once the exp stream ends; Copy shares
                    # the exp table so there is no table reload
                    nc.scalar.activation(ysb, op, AF.Copy, scale=1.0)
                else:
                    nc.vector.tensor_copy(ysb, op)
                # mid-rep y DMAs ride the idle gpsimd queue so sync stays
                # clear for the next rep's wq/x input stream; tail chunks
                # alternate queues for drain throughput
                eng = nc.sync if (tail and ec % 2 == 0) else nc.gpsimd
                eng.dma_start(out=yT_r[ec, :, qsl], in_=ysb)
            return f

        out_emitted[qb] = True
        return [(1.1, mk(ec)) for ec in range(NDC)]

    def sc_chunk(p, qb, kc, es):
        """Both heads' scores for one k-chunk + one exp: the two 64-row
        matmuls sit in opposite PE array halves and run concurrently."""
        kT_t = get_qkT(1, p)
        qT_t = get_qkT(0, p, qb)
        sc_t = bps.tile([128, 2, QB], F32, name="sc", tag="sc", bufs=2)
        for h in range(2):
            nc.tensor.matmul(
                sc_t[:, h, :],
                kT_t[h * 64:(h + 1) * 64, kc * 128:(kc + 1) * 128],
                qT_t[h * 64:(h + 1) * 64, :],
                start=True, stop=True, tile_position=(h * 64, 0))
        nc.scalar.activation(
            es[:, kc, :, :], sc_t, AF.Exp, bias=ebias, scale=SCALE)

    def pv_chunk_fns(p, qb, es):
        """8 PV chunks (head0/head1 alternating, 4 kc each) + drains."""
        ctx_ref = [None, None]

        def mk(h, c):
            def f():
                if c == 0:
                    ctx_ref[h] = bps.tile([128, QB], F32, name="ctx",
                                          tag="ctx", bufs=2)
                hh = p * 2 + h
                for kc in range(4 * c, 4 * c + 4):
                    nc.tensor.matmul(
                        ctx_ref[h][0:HD + 1, :], vt[kc][:, hh, :],
                        es[:, kc, h, :],
                        start=(kc == 0), stop=(kc == NKC - 1))
            return f

        def drain(h):
            def f():
                hh = p * 2 + h
                nc.vector.tensor_copy(
                    ctxU[p][qb][h * 64:(h + 1) * 64, :],
                    ctx_ref[h][0:HD, :])
                db = (hh % 4) * 32
                nc.vector.tensor_copy(
                    den[hh // 4][qb][db:db + 1, :],
                    ctx_ref[h][HD:HD + 1, :])
            return f

        chunks = [mk(h, c) for c in range(4) for h in range(2)]
        return chunks, [drain(0), drain(1)]

    def norm_pieces(qb):
        """Normalize for one q-block as 4 low-PE-cost pieces: two
        reciprocals (spread so the DVE queue never bursts) and two
        scale groups of 4 heads each."""
        rdens = {}

        def recip(half):
            def f():
                r = big.tile([128, QB], BF16, name="rden", tag="rden",
                             bufs=1)
                with nc.allow_low_precision(reason="softmax denom recip"):
                    nc.vector.reciprocal(r, den[half][qb])
                # one strided DMA hops rows {0,32,64,96} down to a
                # base-0 temp (partition_broadcast only reads part. 0)
                d4 = big.tile([1, 4, QB], BF16, name="denr", tag="denr",
                              bufs=2)
                nc.sync.dma_start(out=d4, in_=r[0:128:32, :])
                rdens[half] = d4
            return f

        def scale4(half):
            def f():
                d4 = rdens[half]
                for j in range(4):
                    hh = half * 4 + j
                    p, h = hh // 2, hh % 2
                    # full-height broadcast so the in-place scale reads
                    # both operands at the same base partition
                    denb = big.tile([128, QB], BF16, name="denb",
                                    tag="denb", bufs=1)
                    nc.gpsimd.partition_broadcast(denb, d4[0:1, j, :])
                    nc.vector.tensor_tensor(
                        ctxU[p][qb][h * 64:(h + 1) * 64, :],
                        ctxU[p][qb][h * 64:(h + 1) * 64, :],
                        denb[h * 64:(h + 1) * 64, :], op=MULT)
            return f

        return [recip(0), recip(1), scale4(0), scale4(1)]

    # Slot scheduler: one slot per (pair, q-block). Per slot emit 16
    # paired score chunks (2 MMs + 1 exp each) with the previous slot's
    # 8 PV chunks and filler pieces (V projection at the start, QK
    # projections of the next pair, out-proj chunks at the end) woven
    # between them, so the PE stays busy while ACT digests the exps.
    # preamble: pair-0 projections interleaved with the first half of the
    # V projection — a qk subunit needs ~2.4us of DVE (RoPE) but only
    # ~1.7us of PE, so back-to-back qk subunits starve the PE; the
    # DVE-light V subunits fill the difference.
    for i, args in enumerate(qk_pair(0)):
        qk_subunit(*args)
        v_subunit(i)

    filler_q = [(1.8, (lambda s: lambda: v_subunit(s))(s))
                for s in range(NDC, NKC)]
    aux_q = []      # low-PE-cost pieces (normalize, out-proj push)
    pv_prev, drains_prev = None, None

    slots = [(p, qb) for p in range(NFC) for qb in range(NQB)]
    # per-slot PE filler budgets (us): the first two slots must absorb
    # the whole V projection (PV of slot 0 reads every vt k-chunk during
    # slot 1), later slots pace the next pair's QK projections.
    budgets = {0: 16.0, 1: 14.0, 2: 10.0, 3: 10.0,
               13: 10.0, 14: 10.0, 15: 10.0}
    for si, (p, qb) in enumerate(slots):
        if qb == 0 and p + 1 < NFC:
            filler_q.extend(
                (2.3, (lambda a: lambda: qk_subunit(*a))(a))
                for a in qk_pair(p + 1))
        es = esp.tile([128, NKC, 2, QB], BF16, name="es", tag="es",
                      bufs=2)
        budget = budgets.get(si, 6.0)
        aux_gs = range(8) if p == NFC - 1 else (3, 7)
        for gi in range(8):
            sc_chunk(p, qb, 2 * gi, es)
            sc_chunk(p, qb, 2 * gi + 1, es)
            if pv_prev is not None:
                pv_prev[gi]()
            # up to two filler pieces per score chunk-pair, budget-paced
            for _ in range(2):
                if filler_q and budget > 0:
                    cost, fn = filler_q.pop(0)
                    fn()
                    budget -= cost
            if gi in aux_gs and aux_q:
                aux_q.pop(0)()
        if drains_prev is not None:
            for d in drains_prev:
                d()
        pv_prev, drains_prev = pv_chunk_fns(p, qb, es)
        if p == NFC - 1 and qb > 0:
            # drains for (p, qb-1) just ran; qb-1's normalize + out-proj
            # can be queued.
            aux_q.extend(norm_pieces(qb - 1))

            def mk_push(qb=qb - 1):
                def f():
                    filler_q.extend(out_proj_chunks(qb))
                return f
            aux_q.append(mk_push())
        if p == NFC - 2 and qb == NQB - 1:
            # Slot (3,0) needs all of kT3 plus qT3's first q-block, so
            # flush all but the last NQB-1 fillers (qT3 q-blocks 1..3,
            # k-first qk_pair order); those drip one per (3, qb) slot,
            # feeding the otherwise filler-starved last-pair slots.
            while len(filler_q) > NQB - 1:
                cost, fn = filler_q.pop(0)
                fn()

    # tail: last slot's PV + drains, then remaining normalize + out-proj
    for gi in range(8):
        pv_prev[gi]()
        if filler_q:
            cost, fn = filler_q.pop(0)
            fn()
    for d in drains_prev:
        d()
    for fn in aux_q:
        fn()
    for cost, fn in filler_q:
        fn()
    for qb in range(NQB):
        if not out_emitted[qb]:
            for fn in norm_pieces(qb):
                fn()
            for cost, fn in out_proj_chunks(qb, tail=True):
                fn()


def _rope_tables():
    inv_freq = 1.0 / (10000.0 ** (np.arange(0, HD, 2, dtype=np.float64) / HD))
    pos = np.arange(S, dtype=np.float64)
    sinu = pos[None, :] * inv_freq[:, None]          # [32, S]
    c = np.sin(sinu).astype(np.float32)              # torch code calls this 'cos'
    s = np.cos(sinu).astype(np.float32)              # and this 'sin'
    c2 = np.tile(c, (4, 1))                          # [128, S]
    s2 = np.concatenate([-s, s, -s, s], axis=0)      # [128, S]
    return np.ascontiguousarray(c2), np.ascontiguousarray(s2)


def make_in_maps(inp):
    """inp: dict of full numpy inputs -> list of 8 per-core input maps."""
    import ml_dtypes
    BF = ml_dtypes.bfloat16
    c2, s2 = _rope_tables()
    maps = []
    for c in range(NCORES):
        b, hg = c // 2, c % 2
        fsl = slice(hg * FC, (hg + 1) * FC)
        x = np.asarray(inp["hidden_states"][b], np.float32)
        xp_ = np.ascontiguousarray(
            x.T.reshape(NDC, 128, S).transpose(1, 0, 2)).astype(BF)
        wqp = np.ascontiguousarray(
            np.asarray(inp["Wq"], np.float32)[fsl].T.reshape(NDC, 128, FC)
            .transpose(1, 0, 2)).astype(BF)
        wkp = np.ascontiguousarray(
            np.asarray(inp["Wk"], np.float32)[fsl].T.reshape(NDC, 128, FC)
            .transpose(1, 0, 2)).astype(BF)
        wvp = np.ascontiguousarray(
            np.asarray(inp["Wv"], np.float32)[fsl].T.reshape(NDC, 128, FC)
            .transpose(1, 0, 2)).astype(BF)
        wop = np.ascontiguousarray(
            np.asarray(inp["Wo"], np.float32)[:, fsl].T.reshape(NFC, 128, D)
            .transpose(1, 0, 2)).astype(BF)
        bqp = np.ascontiguousarray(
            np.asarray(inp["bq"], np.float32)[fsl].reshape(NFC, 128).T)
        bkp = np.ascontiguousarray(
            np.asarray(inp["bk"], np.float32)[fsl].reshape(NFC, 128).T)
        bvp = np.ascontiguousarray(np.broadcast_to(
            np.asarray(inp["bv"], np.float32)[fsl][None, :],
            (128, FC))).astype(BF)
        maps.append({
            "xp": xp_, "wq": wqp, "wk": wkp, "wv": wvp, "wo": wop,
            "bq": bqp, "bk": bkp, "bvf": bvp,
            "c2": c2.astype(BF), "s2": s2.astype(BF),
        })
    return maps


_NC_CACHE = {}


def kernel(hidden_states, Wq, bq, Wk, bk, Wv, bv, Wo, bo):
    if "nc" not in _NC_CACHE:
        _NC_CACHE["nc"] = build_kernel()
    nc = _NC_CACHE["nc"]
    in_maps = make_in_maps({
        "hidden_states": hidden_states, "Wq": Wq, "bq": bq, "Wk": Wk, "bk": bk,
        "Wv": Wv, "bv": bv, "Wo": Wo,
    })
    res = run_bass_kernel_spmd(nc, in_maps, list(range(NCORES)))
    bo = np.asarray(bo, np.float32)
    out = np.empty((B, S, D), np.float32)
    for b in range(B):
        acc = (np.asarray(res.results[2 * b]["yT"]).astype(np.float32)
               + np.asarray(res.results[2 * b + 1]["yT"]).astype(np.float32))
        out[b] = acc.T + bo[None, :]
    return out
